# Initial kernel scaffold
#
"""Bass/Trainium2 kernel for nn_BF16AccumConv2d.

Conv2d: x (8, 64, 128, 128) fp32, weight (64, 64, 3, 3) bf16, bias (64,) bf16
-> out (8, 64, 126, 126) bf16 (stride 1, VALID).

Sharding: data-parallel, one image per NeuronCore (8 images, 8 cores).

Per-core layout:
  - x cast to bf16 on host (reference casts x to bf16 before the conv).
  - SBUF x tile [128 partitions, 65, 128]: partitions 0-63 hold input rows
    0..64 (top half + halo), partitions 64-127 hold input rows 63..127.
  - Conv as 9 shifted matmuls (K=C_in=64, M=C_out=64, N=4 out rows * 126)
    accumulating in PSUM.  Four independent PE quadrants run concurrently
    via tile_position (auto-derived from base partitions):
      (array rows 0-63,  psum parts 0-63)  -> top half,    row group A
      (array rows 64-127, psum parts 0-63) -> bottom half, row group A
      (array rows 0-63,  psum parts 64-127)-> top half,    row group B
      (array rows 64-127, psum parts 64-127)-> bottom half, row group B
    Each quadrant accumulates all 9 taps for its own output region, so no
    two streams ever accumulate into the same PSUM elements.
  - Epilogue: activation(Copy, bias per-partition) fp32 PSUM -> bf16 SBUF,
    split between ScalarE and VectorE, then DMA to HBM.
"""

import numpy as np
from ml_dtypes import bfloat16

C_IN, C_OUT = 64, 64
H, W = 128, 128
KH, KW = 3, 3
HO, WO = 126, 126
B = 8
N_CORES = 8
HALF_IN = 65   # input rows per half (top: 0..64, bottom: 63..127)
HALF_OUT = 63  # output rows per half
ROWS_PER_GROUP = 4
N_SUPER = 8    # super-tiles per image; each covers 8 output rows per half

_CACHE = {}


def _build_nc():
    import concourse.bass as bass
    import concourse.mybir as mybir
    from concourse.tile import TileContext

    fp32 = mybir.dt.float32
    bf16 = mybir.dt.bfloat16

    nc = bass.Bass()
    x_ext = nc.declare_dram_parameter("x", [C_IN, H, W], bf16, isOutput=False)
    w_ext = nc.declare_dram_parameter("w", [128, KH * KW, C_OUT], bf16, isOutput=False)
    b_ext = nc.declare_dram_parameter("b", [128, 1], bf16, isOutput=False)
    out_ext = nc.declare_dram_parameter("out", [C_OUT, HO, WO], bf16, isOutput=True)

    with TileContext(nc) as tc:
        with (
            tc.tile_pool(name="const", bufs=1) as cpool,
            tc.tile_pool(name="xin", bufs=1) as xpool,
            tc.tile_pool(name="psum", bufs=2, space="PSUM") as ppool,
            tc.tile_pool(name="outb", bufs=3) as opool,
        ):
            w_sb = cpool.tile([128, KH * KW, C_OUT], bf16)
            nc.sync.dma_start(out=w_sb[:], in_=w_ext[:])
            b_sb = cpool.tile([128, 1], bf16)
            nc.sync.dma_start(out=b_sb[:], in_=b_ext[:])

            xt = xpool.tile([128, HALF_IN, W], bf16)
            nc.sync.dma_start(out=xt[0:64], in_=x_ext[:, 0:HALF_IN, :])
            nc.sync.dma_start(out=xt[64:128], in_=x_ext[:, H - HALF_IN : H, :])

            for g in range(N_SUPER):
                r0 = 8 * g                      # first output row of group A (per half)
                nA = ROWS_PER_GROUP
                nB = min(ROWS_PER_GROUP, HALF_OUT - r0 - nA)  # 4, last tile 3
                rB = r0 + nA

                ps_top = ppool.tile([128, ROWS_PER_GROUP, WO], fp32, tag="pstop")
                ps_bot = ppool.tile([128, ROWS_PER_GROUP, WO], fp32, tag="psbot")

                for t in range(KH * KW):
                    i, j = t // KW, t % KW
                    first, last = t == 0, t == KH * KW - 1
                    # top half, group A  -> quadrant (0, 0)
                    nc.tensor.matmul(
                        out=ps_top[0:64, 0:nA, :],
                        lhsT=w_sb[0:64, t, :],
                        rhs=xt[0:64, r0 + i : r0 + i + nA, j : j + WO],
                        start=first, stop=last,
                    )
                    # bottom half, group A -> quadrant (64, 0)
                    nc.tensor.matmul(
                        out=ps_bot[0:64, 0:nA, :],
                        lhsT=w_sb[64:128, t, :],
                        rhs=xt[64:128, r0 + i : r0 + i + nA, j : j + WO],
                        start=first, stop=last,
                    )
                    # top half, group B -> quadrant (0, 64)
                    nc.tensor.matmul(
                        out=ps_top[64:128, 0:nB, :],
                        lhsT=w_sb[0:64, t, :],
                        rhs=xt[0:64, rB + i : rB + i + nB, j : j + WO],
                        start=first, stop=last,
                    )
                    # bottom half, group B -> quadrant (64, 64)
                    nc.tensor.matmul(
                        out=ps_bot[64:128, 0:nB, :],
                        lhsT=w_sb[64:128, t, :],
                        rhs=xt[64:128, rB + i : rB + i + nB, j : j + WO],
                        start=first, stop=last,
                    )

                ob_top = opool.tile([128, ROWS_PER_GROUP, WO], bf16, tag="obtop")
                ob_bot = opool.tile([128, ROWS_PER_GROUP, WO], bf16, tag="obbot")
                copy_f = mybir.ActivationFunctionType.Copy
                add = mybir.AluOpType.add
                # top tile epilogue on ScalarE, bottom on VectorE
                nc.scalar.activation(
                    out=ob_top[0:64, 0:nA, :], in_=ps_top[0:64, 0:nA, :],
                    func=copy_f, bias=b_sb[0:64, :],
                )
                nc.scalar.activation(
                    out=ob_top[64:128, 0:nB, :], in_=ps_top[64:128, 0:nB, :],
                    func=copy_f, bias=b_sb[64:128, :],
                )
                nc.vector.tensor_scalar(
                    out=ob_bot[0:64, 0:nA, :], in0=ps_bot[0:64, 0:nA, :],
                    scalar1=b_sb[0:64, :], scalar2=None, op0=add,
                )
                nc.vector.tensor_scalar(
                    out=ob_bot[64:128, 0:nB, :], in0=ps_bot[64:128, 0:nB, :],
                    scalar1=b_sb[64:128, :], scalar2=None, op0=add,
                )

                nc.sync.dma_start(
                    out=out_ext[:, r0 : r0 + nA, :], in_=ob_top[0:64, 0:nA, :])
                nc.sync.dma_start(
                    out=out_ext[:, rB : rB + nB, :], in_=ob_top[64:128, 0:nB, :])
                nc.sync.dma_start(
                    out=out_ext[:, HALF_OUT + r0 : HALF_OUT + r0 + nA, :],
                    in_=ob_bot[0:64, 0:nA, :])
                nc.sync.dma_start(
                    out=out_ext[:, HALF_OUT + rB : HALF_OUT + rB + nB, :],
                    in_=ob_bot[64:128, 0:nB, :])

    return nc


def _prep_inputs(x, weight, bias):
    x_bf = np.asarray(x).astype(bfloat16)
    w = np.asarray(weight).astype(bfloat16)  # (co, c, kh, kw)
    b = np.asarray(bias).astype(bfloat16)
    # lhsT layout: [c (partition), tap, co], duplicated across both halves
    w_half = np.ascontiguousarray(w.transpose(1, 2, 3, 0).reshape(C_IN, KH * KW, C_OUT))
    w_host = np.concatenate([w_half, w_half], axis=0)  # [128, 9, 64]
    b_host = np.tile(b.reshape(C_OUT, 1), (2, 1))      # [128, 1]
    return x_bf, w_host, b_host


def kernel(x, weight, bias):
    from concourse.bass_utils import run_bass_kernel_spmd

    nc = _CACHE.get("nc")
    if nc is None:
        nc = _CACHE["nc"] = _build_nc()

    x_bf, w_host, b_host = _prep_inputs(x, weight, bias)
    in_maps = [
        {"x": np.ascontiguousarray(x_bf[core]), "w": w_host, "b": b_host}
        for core in range(N_CORES)
    ]
    res = run_bass_kernel_spmd(nc, in_maps, core_ids=list(range(N_CORES)))
    out = np.stack([np.asarray(res.results[i]["out"]) for i in range(N_CORES)])
    return out


# revision 17
# speedup vs baseline: 1.4821x; 1.4821x over previous
"""Bass/Trainium2 kernel for nn_BF16AccumConv2d.

Conv2d: x (8, 64, 128, 128) fp32, weight (64, 64, 3, 3) bf16, bias (64,) bf16
-> out (8, 64, 126, 126) bf16 (stride 1, VALID).

Sharding: data-parallel, one image per NeuronCore (8 images, 8 cores).

Per-core design:
  - x cast to bf16 AND pre-split on host into [128, 65, 128]: partitions
    0-63 = input rows 0..64 (top half), partitions 64-127 = rows 63..127
    (bottom half).  Loaded in 5 bands (one DMA each) so matmuls start as
    soon as the first band lands.
  - Conv as 9 shifted matmuls (K=C_in=64, M=C_out=64, N=4 out rows * 126)
    accumulating in PSUM.  Four PE quadrants run concurrently via
    tile_position (auto-derived from base partitions):
      (rows 0-63,   psum parts 0-63)   top half,    A-block
      (rows 64-127, psum parts 0-63)   bottom half, A-block
      (rows 0-63,   psum parts 64-127) top half,    B-block
      (rows 64-127, psum parts 64-127) bottom half, B-block
    Each quadrant accumulates all 9 taps for its own output region.
  - Supergroup sizes [8,16,16,16,7] output rows per half: the small first
    group means the PE's cold (HAM-throttled) phase covers only 36 matmuls.
  - Epilogue reproduces the reference's double rounding
    bf16(bf16(sum) + bias): psum->bf16 copy then bias add, split across
    ScalarE/VectorE.
  - Output is written to an internal slot layout out[128, 2, 32, 126]
    ([partition-class, top/bottom, row-slot, w]) with ONE output DMA per
    supergroup on the GpSimd DMA ring (inputs use the Sync ring so the two
    streams don't serialize on one HWDGE ring).  Host reassembles.
"""

import numpy as np
from ml_dtypes import bfloat16

C_IN, C_OUT = 64, 64
H, W = 128, 128
KH, KW = 3, 3
HO, WO = 126, 126
B = 8
N_CORES = 8
HALF_IN = 65     # input rows per half (top: 0..64, bottom: 63..127)
HALF_OUT = 63    # output rows per half
SG_ROWS = [8, 16, 16, 16, 7]          # output rows per half per supergroup
SG_BLKA = [4, 8, 8, 8, 4]             # A-block rows (B-block = rest)
SG_SLOT = [0, 4, 12, 20, 28]          # A/B slot base per supergroup
N_SLOT = 32
BAND_IN = 18     # band tile rows

_CACHE = {}


def _build_nc():
    import concourse.mybir as mybir
    from concourse import bacc
    from concourse.tile import TileContext

    fp32 = mybir.dt.float32
    bf16 = mybir.dt.bfloat16

    nc = bacc.Bacc(None, target_bir_lowering=False)
    x_ext = nc.declare_dram_parameter("x", [128, HALF_IN, W], bf16, isOutput=False)
    w_ext = nc.declare_dram_parameter("w", [128, KH * KW, C_OUT], bf16, isOutput=False)
    b_ext = nc.declare_dram_parameter("b", [128, 1], fp32, isOutput=False)
    out_ext = nc.declare_dram_parameter(
        "out", [128, 2, N_SLOT, WO], bf16, isOutput=True)

    with TileContext(nc) as tc:
        with (
            tc.tile_pool(name="const", bufs=1) as cpool,
            tc.tile_pool(name="xin", bufs=3) as xpool,
            tc.tile_pool(name="psum", bufs=2, space="PSUM") as ppool,
            tc.tile_pool(name="sm", bufs=2) as spool,
            tc.tile_pool(name="outb", bufs=2) as opool,
        ):
            # w/b ride the GpSimd DMA ring so the Sync ring's first issue is
            # band 0 (the matmul-critical load)
            w_sb = cpool.tile([128, KH * KW, C_OUT], bf16)
            nc.scalar.dma_start(out=w_sb[:], in_=w_ext[:])
            b_sb = cpool.tile([128, 1], fp32)
            nc.scalar.dma_start(out=b_sb[:], in_=b_ext[:])

            copy_f = mybir.ActivationFunctionType.Copy
            ident_f = mybir.ActivationFunctionType.Identity
            add = mybir.AluOpType.add

            r0 = 0
            for s, rows_half in enumerate(SG_ROWS):
                n_in = rows_half + 2              # input rows needed
                blkA = SG_BLKA[s]
                nbk = blkA // 4                   # psum banks per block
                slot = SG_SLOT[s]

                band = xpool.tile([128, BAND_IN, W], bf16, tag="band")
                nc.sync.dma_start(
                    out=band[:, 0:n_in, :], in_=x_ext[:, r0 : r0 + n_in, :])

                # psum tiles: P[k] = top (A-bank k | B-bank k),
                #             P[nbk+k] = bottom (A-bank k | B-bank k)
                P = [ppool.tile([128, 4, WO], fp32, tag=f"ps{k}", name=f"ps{k}")
                     for k in range(2 * nbk)]
                # rows per bank: A-banks always 4; last B-bank may be short
                nB = [min(4, rows_half - blkA - 4 * k) for k in range(nbk)]

                for t in range(KH * KW):
                    i, j = t // KW, t % KW
                    first, last = t == 0, t == KH * KW - 1
                    for k in range(nbk):  # bank within block
                        ra = 4 * k + i          # band row of A-bank tap
                        rb = blkA + 4 * k + i   # band row of B-bank tap
                        nb = nB[k]
                        nc.tensor.matmul(
                            out=P[k][0:64, 0:4, :],
                            lhsT=w_sb[0:64, t, :],
                            rhs=band[0:64, ra : ra + 4, j : j + WO],
                            start=first, stop=last)
                        nc.tensor.matmul(
                            out=P[nbk + k][0:64, 0:4, :],
                            lhsT=w_sb[64:128, t, :],
                            rhs=band[64:128, ra : ra + 4, j : j + WO],
                            start=first, stop=last)
                        nc.tensor.matmul(
                            out=P[k][64:128, 0:nb, :],
                            lhsT=w_sb[0:64, t, :],
                            rhs=band[0:64, rb : rb + nb, j : j + WO],
                            start=first, stop=last)
                        nc.tensor.matmul(
                            out=P[nbk + k][64:128, 0:nb, :],
                            lhsT=w_sb[64:128, t, :],
                            rhs=band[64:128, rb : rb + nb, j : j + WO],
                            start=first, stop=last)

                # epilogue: bf16(bf16(sum) + bias), both roundings explicit.
                # ob[:, 0] = top half rows, ob[:, 1] = bottom half rows;
                # partitions 0-63 hold A rows, 64-127 hold B rows.
                ob = opool.tile([128, 2, 8, WO], bf16, tag="ob")
                sm = [spool.tile([128, 4, WO], bf16, tag=f"sm{k}", name=f"sm{k}")
                      for k in range(2 * nbk)]
                # per-half chains so the two halves finish independently:
                # top half entirely on ScalarE, bottom half on VectorE.
                for k in range(nbk):
                    nc.scalar.activation(out=sm[k][:], in_=P[k][:], func=copy_f)
                    nc.scalar.activation(
                        out=ob[:, 0, 4 * k : 4 * k + 4, :], in_=sm[k][:],
                        func=ident_f, bias=b_sb[:])
                    nc.vector.tensor_copy(out=sm[nbk + k][:], in_=P[nbk + k][:])
                    nc.vector.tensor_scalar(
                        out=ob[:, 1, 4 * k : 4 * k + 4, :], in0=sm[nbk + k][:],
                        scalar1=b_sb[:], scalar2=None, op0=add)

                # output DMAs per half on the Sync HWDGE ring (idle once the
                # input bands are issued)
                nc.sync.dma_start(
                    out=out_ext[:, 0, slot : slot + blkA, :],
                    in_=ob[:, 0, 0:blkA, :])
                nc.sync.dma_start(
                    out=out_ext[:, 1, slot : slot + blkA, :],
                    in_=ob[:, 1, 0:blkA, :])
                r0 += rows_half

    nc.finalize()
    return nc


def _prep_inputs(x, weight, bias):
    x_bf = np.asarray(x).astype(bfloat16)
    # pre-split halves: [B, 128, 65, 128]
    x_split = np.concatenate(
        [x_bf[:, :, 0:HALF_IN, :], x_bf[:, :, H - HALF_IN : H, :]], axis=1)
    w = np.asarray(weight).astype(bfloat16)  # (co, c, kh, kw)
    b = np.asarray(bias).astype(bfloat16)
    # lhsT layout: [c (partition), tap, co], duplicated across both halves
    w_half = np.ascontiguousarray(w.transpose(1, 2, 3, 0).reshape(C_IN, KH * KW, C_OUT))
    w_host = np.concatenate([w_half, w_half], axis=0)  # [128, 9, 64]
    b_host = np.tile(b.reshape(C_OUT, 1), (2, 1)).astype(np.float32)  # [128, 1]
    return x_split, w_host, b_host


def _reassemble(buf):
    """buf: [128, 2, N_SLOT, WO] -> [C_OUT, HO, WO]."""
    out = np.empty((C_OUT, HO, WO), dtype=buf.dtype)
    r0 = 0
    for s, rows_half in enumerate(SG_ROWS):
        blkA = SG_BLKA[s]
        nb = rows_half - blkA
        slot = SG_SLOT[s]
        for h in range(2):  # top / bottom
            base = r0 + h * HALF_OUT
            out[:, base : base + blkA, :] = buf[0:64, h, slot : slot + blkA, :]
            out[:, base + blkA : base + rows_half, :] = \
                buf[64:128, h, slot : slot + nb, :]
        r0 += rows_half
    return out


def kernel(x, weight, bias):
    from concourse.bass_utils import run_bass_kernel_spmd

    nc = _CACHE.get("nc")
    if nc is None:
        nc = _CACHE["nc"] = _build_nc()

    x_split, w_host, b_host = _prep_inputs(x, weight, bias)
    in_maps = [
        {"x": np.ascontiguousarray(x_split[core]), "w": w_host, "b": b_host}
        for core in range(N_CORES)
    ]
    res = run_bass_kernel_spmd(nc, in_maps, core_ids=list(range(N_CORES)))
    out = np.stack([_reassemble(np.asarray(res.results[i]["out"]))
                    for i in range(N_CORES)])
    return out
